# revision 66
# baseline (speedup 1.0000x reference)
"""TRN2 Bass kernel for nn_Attention_20444044329649.

GroupNorm(32) -> qkv dense -> single-head spatial attention (1024 pos) ->
out dense -> residual.  B=32 examples sharded 4-per-core across 8 cores;
params replicated.

v4 — fp8 DoubleRow main pipeline + phase-shifted stats (vs v3: 249us ->
~209us):

  * scores:  S*isq = Z M' Z^T with M' = isq*Wq Wk^T host-precomputed (kept
    bf16 — fp8 M' was measured as the dominant systematic error source).
    G^T = M'^T Z^T in bf16; S^T = Z_j M'^T Z^T via fp8 DoubleRow
    (zt8 x gt8) — half the matmul instructions of the bf16 path.
  * V' = Z Wv' via fp8 DoubleRow (zt8 x wvp8, host-folded Wv*W_out*16;
    the *16 restores fp8 range, undone via the ones2=16 denominator).
  * O = A V via fp8 DoubleRow; denominators via M=1 ones-lhsT DoubleRow
    matmuls emitted two j's after their exps (never head-block the PE
    FIFO on the ACT), last pair inside the V phase together with the
    row->column DRAM bounce so rc lands before the O phase.
  * phase-shifted stats: transposes for bi+1 interleave into G-phase(bi)
    (PSUM tiles from the shared pm pool); bn_stats/aggr/pool chain runs
    during ST(bi) via hooks (rstd at j==2, affine at j==4, zt16 at
    j==6); zt8 copies on ACT during the O window.  The PE never waits
    on the serial stats chain at example boundaries (v3 lost
    ~6.8us/example to HAM re-throttle there).
  * rstd = exp(-0.5*ln(var+eps)) — Ln/Exp share an ACT table set with
    Exp/Copy/Identity, unlike Sqrt (1.28us table reload per switch).
  * all PSUM compute tiles are [128,1024] (2 banks) -> 1024-wide
    exp/copies halve the ACT instruction-overhead (352 cyc/op).
  * stats matmuls (group pool/expand) in bf16 single-pass (no fp32
    LOW_HIGH double-pass); warmups bf16.
  * engine-queue discipline: x loads + out stores on the sync queue,
    denominator bounce on the (otherwise idle) pool queue, no DMA ever
    on the ACT queue mid-pipeline; final example drains stt across
    DVE and ACT+pool in parallel.
  * b_res == 0 fast path: skips the 32 pool residual-bias adds.
"""

import numpy as np

import concourse.bass as bass
import concourse.mybir as mybir
import concourse.tile as tile
from concourse import bacc
from concourse.bass_utils import run_bass_kernel_spmd
from concourse.masks import make_identity

B, H, W, C = 32, 32, 32, 512
N = H * W                      # 1024 positions
G = 32                         # groups
GS = C // G                    # 16 channels per group
EPS = 1e-5
NCORES = 8
BPC = B // NCORES              # 4 examples per core
ISQ = float(1.0 / np.sqrt(C))  # score scale (folded into M' on host)

F32 = mybir.dt.float32
BF16 = mybir.dt.bfloat16
FP8 = mybir.dt.float8e4
AF = mybir.ActivationFunctionType
ALU = mybir.AluOpType
MS = bass.MemorySpace
DR = mybir.MatmulPerfMode.DoubleRow

SCALE_M = 256.0                # host upscale on M'; exp scale undoes it
SCALE_V = 16.0                 # host upscale on Wv' for fp8 range; folded
                               # into the softmax denominator via ones2
EXP_B = -2.0                   # exp range-compression bias (cancels in
                               # softmax)


class Ctx:
    pass


def _load_x(g, bi, qs):
    xn = g.xn_p.tile([128, 8, 512], F32, tag="xn", name=f"xn{bi}")
    for d in range(8):
        qs[d % len(qs)].dma_start(xn[:, d, :], g.xr[bi, :, d, :])
    return xn


def _load_xb(g, bi, qs):
    """bf16 copy of x (host-precast): transpose source only — half the
    bytes of xn and the only input the next example's PE work waits on."""
    xb = g.xb_p.tile([128, 8, 512], BF16, tag="xb", name=f"xbt{bi}")
    for d in range(8):
        qs[d % len(qs)].dma_start(xb[:, d, :], g.xbr[bi, :, d, :])
    return xb


def _tr_group(g, bi, xb, xt, st6, t):
    """Transpose one channel-chunk t: 8 bf16 PE transposes (single-pass,
    host-precast x) into one PSUM bank, one 1024-wide copy out, two
    bn_stats."""
    nc = g.nc
    trps = g.pm.tile([128, 1024], BF16, tag="pm", name=f"trp{bi}_{t}")
    for i in range(8):
        nc.tensor.matmul(
            trps[:, i * 128:(i + 1) * 128],
            xb[:, i, t * 128:(t + 1) * 128],
            g.ident,
            is_transpose=True,
            start=(i == 0),
            stop=(i == 7),
        )
    nc.vector.tensor_copy(xt[:, t, :], trps)
    for hh in range(2):
        nc.vector.bn_stats(st6[:, t, hh, :],
                           xt[:, t, hh * 512:(hh + 1) * 512])


def _stats_a(g, bi, st6):
    """aggr -> m2 -> group-pool matmul -> variance (DVE/PE only; the ACT
    part is split off so it never head-blocks the exp stream)."""
    nc = g.nc
    mv = g.small.tile([128, 4, 2], F32, tag="mv", name=f"mv{bi}")
    for t in range(4):
        nc.vector.bn_aggr(mv[:, t, :], st6[:, t, :, :])
    m2 = g.small.tile([128, 4, 2], BF16, tag="m2", name=f"m2{bi}")
    mm = g.small.tile([128, 4, 1], F32, tag="mm", name=f"mm{bi}")
    nc.vector.tensor_copy(m2[:, :, 0:1], mv[:, :, 0:1])
    nc.vector.tensor_mul(mm, mv[:, :, 0:1], mv[:, :, 0:1])
    nc.vector.tensor_add(m2[:, :, 1:2], mm, mv[:, :, 1:2])
    ps_g = g.aux.tile([8, 4, 2], F32, tag="aux", name=f"ps_g{bi}")
    nc.tensor.matmul(ps_g, g.a_pool, m2, start=True, stop=True)
    pg = g.small.tile([8, 4, 2], F32, tag="pg", name=f"pg{bi}")
    nc.vector.tensor_copy(pg, ps_g)
    vr = g.small.tile([8, 4, 1], F32, tag="vr", name=f"vr{bi}")
    nc.vector.tensor_mul(vr, pg[:, :, 0:1], pg[:, :, 0:1])
    nc.vector.tensor_sub(vr, pg[:, :, 1:2], vr)
    return pg, vr


def _stats_rstd(g, bi, pg, vr):
    """rstd = exp(-0.5*ln(var+eps)) — Ln/Exp live in the same ACT table
    set as Exp/Copy/Identity, unlike Sqrt (1.28us reload per switch)."""
    nc = g.nc
    gab = g.small.tile([8, 4, 2], BF16, tag="gab", name=f"gab{bi}")
    nc.scalar.activation(vr, vr, AF.Ln, bias=g.eps_c[:8])
    nc.scalar.activation(gab[:, :, 0:1], vr, AF.Exp, scale=-0.5)
    nc.vector.tensor_copy(gab[:, :, 1:2], pg[:, :, 0:1])
    return gab


def _stats_b(g, bi, gab):
    """Expand group stats to channels + affine coefficients."""
    nc = g.nc
    ps_ab = g.aux.tile([128, 4, 2], F32, tag="aux", name=f"ps_ab{bi}")
    nc.tensor.matmul(ps_ab, g.e8, gab, start=True, stop=True)
    ab = g.small.tile([128, 4, 2], F32, tag="ab", name=f"ab{bi}")
    tmpc = g.small.tile([128, 4, 1], F32, tag="tmpc", name=f"tmpc{bi}")
    nc.vector.tensor_mul(ab[:, :, 0:1], ps_ab[:, :, 0:1], g.gns_sb[:, :, 0:1])
    nc.vector.tensor_mul(tmpc, ps_ab[:, :, 1:2], ab[:, :, 0:1])
    nc.vector.tensor_sub(ab[:, :, 1:2], g.gnb_sb[:, :, 0:1], tmpc)
    return ab


def _zt16(g, bi, xt, ab):
    # all on DVE: the ST window's ACT is saturated by exp, DVE is idle
    zt16 = g.zt16_p.tile([128, 4, 1024], BF16, tag="zt16", name=f"zt16_{bi}")
    nc = g.nc
    for t in range(4):
        nc.vector.tensor_scalar(
            out=zt16[:, t, :], in0=xt[:, t, :],
            scalar1=ab[:, t, 0:1], scalar2=ab[:, t, 1:2],
            op0=ALU.mult, op1=ALU.add,
        )
    return zt16


def _zt8_part(g, bi, zt16, zt8, ts, eng=None):
    if zt8 is None:
        zt8 = g.zt8_p.tile([128, 4, 1024], FP8, tag="zt8", name=f"zt8_{bi}")
    for t in ts:
        if eng == "act":
            g.nc.scalar.copy(zt8[:, t, :], zt16[:, t, :])
        else:
            g.nc.vector.tensor_copy(zt8[:, t, :], zt16[:, t, :])
    return zt8


def _g_phase(g, bi, zt16, tr=None):
    """G^T = M'^T Z^T in bf16; interleave next example's transposes."""
    nc = g.nc
    gt = g.gt_p.tile([128, 4, 1024], FP8, tag="gt", name=f"gt{bi}")
    for m in range(4):
        ps = g.pm.tile([128, 1024], F32, tag="pm", name=f"ps_g{bi}_{m}")
        for kk in range(4):
            for h in range(2):
                nc.tensor.matmul(
                    ps[:, h * 512:(h + 1) * 512],
                    g.mq_sb[:, kk, m * 128:(m + 1) * 128],
                    zt16[:, kk, h * 512:(h + 1) * 512],
                    start=(kk == 0),
                    stop=(kk == 3),
                )
        nc.scalar.copy(gt[:, m, :], ps)
        if tr is not None:
            tr(m)
    return gt


def _u_stage(g, bi, zt16):
    """Per-key bias u_j = uvec . z_j  (only when b_qkv != 0)."""
    nc = g.nc
    ps_u = g.aux.tile([128, 8], F32, tag="aux", name=f"ps_u{bi}")
    for j in range(8):
        for kk in range(4):
            nc.tensor.matmul(
                ps_u[:, j:j + 1],
                zt16[:, kk, j * 128:(j + 1) * 128],
                g.uv_sb[:, kk:kk + 1],
                start=(kk == 0),
                stop=(kk == 3),
            )
    u_sb = g.small.tile([128, 8], F32, tag="u_sb", name=f"u_sb{bi}")
    nc.vector.tensor_scalar(out=u_sb, in0=ps_u, scalar1=1.0, scalar2=EXP_B,
                            op0=ALU.mult, op1=ALU.add)
    return u_sb


def _st_phase(g, bi, zt8, gt, u_sb=None, hooks=None):
    """S^T + exp -> ET via fp8 DoubleRow; denominators via M=1 DR matmuls;
    hooks emit the next example's stats/zt work mid-loop."""
    nc = g.nc
    hooks = hooks or {}
    et = g.et_p.tile([128, 8, 1024], FP8, tag="et", name=f"et{bi}")
    s_ps = g.sden.tile([1, 2, 512], F32, tag="sden", name=f"sps{bi}")
    for j in range(8):
        ps = g.pm.tile([128, 1024], F32, tag="pm", name=f"ps_s{bi}_{j}")
        for k2 in range(2):
            for h in range(2):
                nc.tensor.matmul(
                    ps[:, h * 512:(h + 1) * 512],
                    zt8[:, 2 * k2:2 * k2 + 2, j * 128:(j + 1) * 128],
                    gt[:, 2 * k2:2 * k2 + 2, h * 512:(h + 1) * 512],
                    start=(k2 == 0),
                    stop=(k2 == 1),
                    perf_mode=DR,
                )
        nc.scalar.activation(
            et[:, j, :], ps, AF.Exp,
            scale=1.0 / SCALE_M,
            bias=g.neg2 if u_sb is None else u_sb[:, j:j + 1])
        # denominator for pair jj emitted two j's after its exps complete,
        # so these matmuls never head-block the PE FIFO on the ACT
        if j in (3, 5, 7):
            jj = (j - 3) // 2
            for h in range(2):
                nc.tensor.matmul(
                    s_ps[:, h, :],
                    g.ones2[:, :, 0:1],
                    et[:, 2 * jj:2 * jj + 2, h * 512:(h + 1) * 512],
                    start=(jj == 0),
                    stop=False,
                    perf_mode=DR,
                )
        if j in hooks:
            hooks[j]()
    return et, s_ps


def _v_phase(g, bi, zt8, after_p0=None):
    """V' = Z Wv' via fp8 DoubleRow; 512-wide fp8 copies alternating
    DVE/ACT so the last copy lags the last fill minimally."""
    nc = g.nc
    v = g.v_p.tile([128, 8, 512], FP8, tag="v", name=f"v{bi}")
    for p in range(4):
        ps = g.pm.tile([128, 1024], F32, tag="pm", name=f"ps_v{bi}_{p}")
        for k2 in range(2):
            for io in range(2):
                i = 2 * p + io
                nc.tensor.matmul(
                    ps[:, io * 512:(io + 1) * 512],
                    zt8[:, 2 * k2:2 * k2 + 2, i * 128:(i + 1) * 128],
                    g.wvp_sb[:, 2 * k2:2 * k2 + 2, :],
                    start=(k2 == 0),
                    stop=(k2 == 1),
                    perf_mode=DR,
                )
        for io in range(2):
            i = 2 * p + io
            if io == 0:
                nc.vector.tensor_copy(v[:, i, :], ps[:, io * 512:(io + 1) * 512])
            else:
                nc.scalar.copy(v[:, i, :], ps[:, io * 512:(io + 1) * 512])
        if p == 0 and after_p0 is not None:
            after_p0()
    return v


def _denom_last(g, bi, s_ps, et):
    """Final denominator pair (jj=3) plus the row->column DRAM bounce,
    emitted inside the V phase so rc is back before the O phase needs
    it."""
    nc = g.nc
    for h in range(2):
        nc.tensor.matmul(
            s_ps[:, h, :],
            g.ones2[:, :, 0:1],
            et[:, 6:8, h * 512:(h + 1) * 512],
            start=False,
            stop=True,
            perf_mode=DR,
        )
    s_sb = g.small.tile([1, 1024], F32, tag="s_sb", name=f"s_sb{bi}")
    for h in range(2):
        nc.vector.tensor_copy(s_sb[:, h * 512:(h + 1) * 512], s_ps[:, h, :])
    s_dram = g.dram.tile([1, 1024], F32, tag="s_dram", name=f"s_dram{bi}")
    nc.gpsimd.dma_start(s_dram, s_sb)
    s_col = g.small.tile([128, 8], F32, tag="s_col", name=f"s_col{bi}")
    nc.gpsimd.dma_start(s_col, s_dram.rearrange("o (t p) -> p (o t)", p=128))
    return s_col


def _denom_fin(g, bi, s_col):
    rc = g.small.tile([128, 8], F32, tag="rc", name=f"rc{bi}")
    g.nc.vector.reciprocal(rc, s_col)
    return rc


def _o_phase(g, bi, xn, v, et, rc, has_b, last=False):
    """O natural via fp8 DoubleRow, residual stt, store."""
    nc = g.nc
    res = g.res_p.tile([128, 8, 512], F32, tag="res", name=f"res{bi}")
    out_q = [nc.sync] if not last else [nc.sync, nc.scalar, nc.gpsimd]
    if has_b:
        for i in range(8):
            nc.gpsimd.tensor_add(xn[:, i, :], xn[:, i, :], g.bres_bc)
    for p in range(4):
        ps = g.pm.tile([128, 1024], F32, tag="pm", name=f"ps_o{bi}_{p}")
        for io in range(2):
            i = 2 * p + io
            for jj in range(4):
                nc.tensor.matmul(
                    ps[:, io * 512:(io + 1) * 512],
                    et[:, 2 * jj:2 * jj + 2, i * 128:(i + 1) * 128],
                    v[:, 2 * jj:2 * jj + 2, :],
                    start=(jj == 0),
                    stop=(jj == 3),
                    perf_mode=DR,
                )
        for io in range(2):
            i = 2 * p + io
            if last and io == 1:
                # final-example drain, 3-way: ACT scales; the residual
                # add goes to the pool for the early chunks and to the
                # DVE for the late ones (a pool add is 1.27us vs 0.74
                # on DVE — the serial pool chain was the old tail)
                nc.scalar.activation(res[:, i, :],
                                     ps[:, io * 512:(io + 1) * 512],
                                     AF.Identity, scale=rc[:, i:i + 1])
                if p < 2:
                    nc.gpsimd.tensor_add(res[:, i, :], res[:, i, :],
                                         xn[:, i, :])
                else:
                    nc.vector.tensor_add(res[:, i, :], res[:, i, :],
                                         xn[:, i, :])
            else:
                nc.vector.scalar_tensor_tensor(
                    out=res[:, i, :], in0=ps[:, io * 512:(io + 1) * 512],
                    scalar=rc[:, i:i + 1],
                    in1=xn[:, i, :], op0=ALU.mult, op1=ALU.add,
                )
            out_q[i % len(out_q)].dma_start(g.outr[bi, :, i, :], res[:, i, :])


def build_program(has_u, has_b):
    nc = bacc.Bacc("TRN2", target_bir_lowering=False, debug=False)

    x_d = nc.dram_tensor("x", [BPC, N, C], F32, kind="ExternalInput")
    xb_d = nc.dram_tensor("xb16", [BPC, N, C], BF16, kind="ExternalInput")
    mq_d = nc.dram_tensor("m_qk", [C, C], BF16, kind="ExternalInput")
    wvp_d = nc.dram_tensor("w_vp", [C, C], FP8, kind="ExternalInput")
    gns_d = nc.dram_tensor("gn_scale", [C], F32, kind="ExternalInput")
    gnb_d = nc.dram_tensor("gn_bias", [C], F32, kind="ExternalInput")
    if has_b:
        bres_d = nc.dram_tensor("b_res", [C], F32, kind="ExternalInput")
    if has_u:
        uv_d = nc.dram_tensor("u_vec", [C], F32, kind="ExternalInput")
    out_d = nc.dram_tensor("out", [BPC, N, C], F32, kind="ExternalOutput")

    g = Ctx()
    g.nc = nc
    g.xr = x_d.ap().rearrange("b (i p) c -> b p i c", p=128)
    g.xbr = xb_d.ap().rearrange("b (i p) c -> b p i c", p=128)
    g.outr = out_d.ap().rearrange("b (i p) c -> b p i c", p=128)

    with tile.TileContext(nc) as tc:
        from contextlib import ExitStack
        with ExitStack() as ctx:
            const = ctx.enter_context(tc.tile_pool(name="const", bufs=1))
            g.pm = ctx.enter_context(tc.tile_pool(name="pm", bufs=2, space=MS.PSUM))
            g.sden = ctx.enter_context(tc.tile_pool(name="sden", bufs=1, space=MS.PSUM))
            g.aux = ctx.enter_context(tc.tile_pool(name="aux", bufs=2, space=MS.PSUM))
            g.xn_p = ctx.enter_context(tc.tile_pool(name="xn", bufs=3))
            g.xb_p = ctx.enter_context(tc.tile_pool(name="xb", bufs=3))
            g.xt_p = ctx.enter_context(tc.tile_pool(name="xtp", bufs=2))
            g.zt16_p = ctx.enter_context(tc.tile_pool(name="zt16p", bufs=2))
            g.zt8_p = ctx.enter_context(tc.tile_pool(name="zt8p", bufs=2))
            g.gt_p = ctx.enter_context(tc.tile_pool(name="gtp", bufs=2))
            g.v_p = ctx.enter_context(tc.tile_pool(name="vp", bufs=2))
            g.et_p = ctx.enter_context(tc.tile_pool(name="etp", bufs=2))
            g.res_p = ctx.enter_context(tc.tile_pool(name="resp", bufs=1))
            g.small = ctx.enter_context(tc.tile_pool(name="small", bufs=3))
            g.dram = ctx.enter_context(tc.tile_pool(name="dram", bufs=2,
                                                    space=MS.DRAM))

            # ---- input DMA for examples 0 and 1 first: critical path ----
            xb0 = _load_xb(g, 0, [nc.sync, nc.scalar, nc.gpsimd])
            xb1 = _load_xb(g, 1, [nc.gpsimd, nc.sync, nc.scalar])
            xn0 = _load_x(g, 0, [nc.sync, nc.scalar, nc.gpsimd])
            xn1 = _load_x(g, 1, [nc.gpsimd, nc.sync, nc.scalar])

            # ---- constants ----------------------------------------------
            g.ident = const.tile([128, 128], BF16)
            make_identity(nc, g.ident)
            g.identf = const.tile([128, 128], F32)
            make_identity(nc, g.identf)
            g.junk = const.tile([128, 512], BF16)
            nc.vector.memset(g.junk, 0.5)

            # PE warmup: bf16 matmuls with no DMA dependency, issued while
            # the input DMAs run, so the HAM clock gate reaches K=8/8
            # before the first productive matmul.
            def warm(n, salt=[0]):
                for _ in range(n):
                    salt[0] += 1
                    ps_w = g.pm.tile([128, 1024], F32, tag="pm",
                                     name=f"ps_w{salt[0]}")
                    nc.tensor.matmul(ps_w[:, 0:512], g.ident, g.junk,
                                     start=True, stop=True)
            g.warm = warm
            warm(10)

            g.a_pool = const.tile([128, 8], BF16)
            nc.gpsimd.memset(g.a_pool, 1.0 / GS)
            nc.gpsimd.affine_select(
                out=g.a_pool, in_=g.a_pool, compare_op=ALU.is_ge, fill=0.0,
                base=0, pattern=[[-GS, 8]], channel_multiplier=1)
            nc.gpsimd.affine_select(
                out=g.a_pool, in_=g.a_pool, compare_op=ALU.is_ge, fill=0.0,
                base=GS - 1, pattern=[[GS, 8]], channel_multiplier=-1)

            g.e8 = const.tile([8, 128], BF16)
            nc.gpsimd.memset(g.e8, 1.0)
            nc.gpsimd.affine_select(
                out=g.e8, in_=g.e8, compare_op=ALU.is_ge, fill=0.0,
                base=0, pattern=[[1, 128]], channel_multiplier=-GS)
            nc.gpsimd.affine_select(
                out=g.e8, in_=g.e8, compare_op=ALU.is_ge, fill=0.0,
                base=GS - 1, pattern=[[-1, 128]], channel_multiplier=GS)

            g.ones2 = const.tile([128, 2, 16], FP8)
            nc.vector.memset(g.ones2, SCALE_V)
            g.neg2 = const.tile([128, 1], F32)
            nc.vector.memset(g.neg2, EXP_B)
            g.eps_c = const.tile([128, 1], F32)
            nc.vector.memset(g.eps_c, EPS)

            g.mq_sb = const.tile([128, 4, C], BF16)
            mqr = mq_d.ap().rearrange("(t p) d -> t p d", p=128)
            for t in range(4):
                nc.gpsimd.dma_start(g.mq_sb[:, t, :], mqr[t])
            g.wvp_sb = const.tile([128, 4, C], FP8)
            wvr = wvp_d.ap().rearrange("(t p) d -> t p d", p=128)
            for t in range(4):
                nc.gpsimd.dma_start(g.wvp_sb[:, t, :], wvr[t])

            g.gns_sb = const.tile([128, 4, 1], F32)
            nc.sync.dma_start(g.gns_sb[:, :, 0:1],
                              gns_d.ap().rearrange("(t p) -> p t", p=128))
            g.gnb_sb = const.tile([128, 4, 1], F32)
            nc.sync.dma_start(g.gnb_sb[:, :, 0:1],
                              gnb_d.ap().rearrange("(t p) -> p t", p=128))

            def bcast(src_ap):
                return bass.AP(
                    tensor=src_ap.tensor, offset=src_ap.offset,
                    ap=[[0, 128]] + [list(p) for p in src_ap.ap])

            if has_b:
                g.bres_bc = const.tile([128, 512], F32)
                nc.gpsimd.dma_start(g.bres_bc, bcast(bres_d.ap()))

            if has_u:
                uvf = const.tile([128, 4], F32)
                nc.sync.dma_start(uvf, uv_d.ap().rearrange("(t p) -> p t", p=128))
                g.uv_sb = const.tile([128, 4], BF16)
                nc.vector.tensor_copy(g.uv_sb, uvf)

            # ---- example-0 prologue -------------------------------------
            warm(40)
            xt0 = g.xt_p.tile([128, 4, 1024], BF16, tag="xt", name="xt0")
            st6_0 = g.small.tile([128, 4, 2, 6], F32, tag="st6", name="st6_0")
            for t in range(4):
                _tr_group(g, 0, xb0, xt0, st6_0, t)
            warm(8)
            pg0, vr0 = _stats_a(g, 0, st6_0)
            gab0 = _stats_rstd(g, 0, pg0, vr0)
            warm(14)
            ab0 = _stats_b(g, 0, gab0)
            warm(10)
            zt16_0 = _zt16(g, 0, xt0, ab0)
            warm(10)
            zt8_0 = _zt8_part(g, 0, zt16_0, None, (0, 1, 2, 3))
            warm(6)

            # ---- pipelined per-example emission -------------------------
            xns = {0: xn0, 1: xn1}
            xbs = {0: xb0, 1: xb1}
            state = (zt16_0, zt8_0)
            nxt = {}
            for bi in range(BPC):
                zt16, zt8 = state
                xn = xns[bi]
                if bi + 2 < BPC:
                    xbs[bi + 2] = _load_xb(g, bi + 2, [nc.gpsimd])
                    xns[bi + 2] = _load_x(g, bi + 2, [nc.sync])
                tr = None
                hooks = {}
                if bi + 1 < BPC:
                    xb1_t = xbs[bi + 1]
                    xt1 = g.xt_p.tile([128, 4, 1024], BF16, tag="xt",
                                      name=f"xt{bi+1}")
                    st6 = g.small.tile([128, 4, 2, 6], F32, tag="st6",
                                       name=f"st6_{bi+1}")

                    def tr(m, xb1_t=xb1_t, xt1=xt1, st6=st6, b1=bi + 1):
                        _tr_group(g, b1, xb1_t, xt1, st6, m)
                gt = _g_phase(g, bi, zt16, tr)
                if bi + 1 < BPC:
                    nxt["pgvr"] = _stats_a(g, bi + 1, st6)

                    def hook2(b1=bi + 1):
                        nxt["gab"] = _stats_rstd(g, b1, *nxt["pgvr"])

                    def hook4(b1=bi + 1):
                        nxt["ab"] = _stats_b(g, b1, nxt["gab"])

                    def hook6(xt1=xt1, b1=bi + 1):
                        nxt["zt16"] = _zt16(g, b1, xt1, nxt["ab"])
                    hooks = {2: hook2, 4: hook4, 6: hook6}
                u_sb = _u_stage(g, bi, zt16) if has_u else None
                et, s_ps = _st_phase(g, bi, zt8, gt, u_sb, hooks)
                box = {}

                def after_p0(box=box, s_ps=s_ps, et=et, b=bi):
                    box["s_col"] = _denom_last(g, b, s_ps, et)
                v = _v_phase(g, bi, zt8, after_p0=after_p0)
                rc = _denom_fin(g, bi, box["s_col"])
                if bi + 1 < BPC:
                    nxt["zt8"] = _zt8_part(g, bi + 1, nxt["zt16"],
                                           None, (0, 1, 2, 3), eng="act")
                _o_phase(g, bi, xn, v, et, rc, has_b, last=(bi == BPC - 1))
                if bi + 1 < BPC:
                    state = (nxt["zt16"], nxt["zt8"])

    nc.compile()
    return nc


_NC = {}


def _get_nc(has_u, has_b):
    key = (has_u, has_b)
    if key not in _NC:
        _NC[key] = build_program(has_u, has_b)
    return _NC[key]


def kernel(x, t, gn_scale, gn_bias, w_qkv, b_qkv, w_out, b_out):
    import ml_dtypes
    x = np.ascontiguousarray(np.asarray(x, np.float32).reshape(B, N, C))
    w_qkv = np.asarray(w_qkv, np.float32)
    b_qkv = np.asarray(b_qkv, np.float32)
    w_out = np.asarray(w_out, np.float32)
    b_out = np.asarray(b_out, np.float32)
    wq, wk, wv = w_qkv[:, 0:C], w_qkv[:, C:2 * C], w_qkv[:, 2 * C:3 * C]
    bq, bv = b_qkv[0:C], b_qkv[2 * C:3 * C]

    m_qk = ((wq @ wk.T) * (ISQ * SCALE_M)).astype(ml_dtypes.bfloat16)
    w_vp = np.clip((wv @ w_out) * SCALE_V, -240.0, 240.0).astype(
        ml_dtypes.float8_e4m3)
    b_res = (bv @ w_out + b_out).astype(np.float32)
    u_vec = ((wk @ bq) * ISQ).astype(np.float32)
    has_u = bool(np.any(u_vec != 0.0))
    has_b = bool(np.any(b_res != 0.0))

    shared = {
        "m_qk": np.ascontiguousarray(m_qk),
        "w_vp": np.ascontiguousarray(w_vp),
        "gn_scale": np.ascontiguousarray(np.asarray(gn_scale, np.float32)),
        "gn_bias": np.ascontiguousarray(np.asarray(gn_bias, np.float32)),
    }
    if has_b:
        shared["b_res"] = np.ascontiguousarray(b_res)
    if has_u:
        shared["u_vec"] = np.ascontiguousarray(u_vec)
    xb16 = x.astype(ml_dtypes.bfloat16)
    in_maps = [
        {"x": x[c * BPC:(c + 1) * BPC], "xb16": xb16[c * BPC:(c + 1) * BPC],
         **shared} for c in range(NCORES)
    ]
    nc = _get_nc(has_u, has_b)
    res = run_bass_kernel_spmd(nc, in_maps, core_ids=list(range(NCORES)))
    out = np.concatenate([res.results[c]["out"] for c in range(NCORES)], axis=0)
    return out.reshape(B, H, W, C)


# revision 67
# speedup vs baseline: 1.0004x; 1.0004x over previous
"""TRN2 Bass kernel for nn_Attention_20444044329649.

GroupNorm(32) -> qkv dense -> single-head spatial attention (1024 pos) ->
out dense -> residual.  B=32 examples sharded 4-per-core across 8 cores;
params replicated.

v4 — fp8 DoubleRow main pipeline + phase-shifted stats (vs v3: 249us ->
~209us):

  * scores:  S*isq = Z M' Z^T with M' = isq*Wq Wk^T host-precomputed (kept
    bf16 — fp8 M' was measured as the dominant systematic error source).
    G^T = M'^T Z^T in bf16; S^T = Z_j M'^T Z^T via fp8 DoubleRow
    (zt8 x gt8) — half the matmul instructions of the bf16 path.
  * V' = Z Wv' via fp8 DoubleRow (zt8 x wvp8, host-folded Wv*W_out*16;
    the *16 restores fp8 range, undone via the ones2=16 denominator).
  * O = A V via fp8 DoubleRow; denominators via M=1 ones-lhsT DoubleRow
    matmuls emitted two j's after their exps (never head-block the PE
    FIFO on the ACT), last pair inside the V phase together with the
    row->column DRAM bounce so rc lands before the O phase.
  * phase-shifted stats: transposes for bi+1 interleave into G-phase(bi)
    (PSUM tiles from the shared pm pool); bn_stats/aggr/pool chain runs
    during ST(bi) via hooks (rstd at j==2, affine at j==4, zt16 at
    j==6); zt8 copies on ACT during the O window.  The PE never waits
    on the serial stats chain at example boundaries (v3 lost
    ~6.8us/example to HAM re-throttle there).
  * rstd = exp(-0.5*ln(var+eps)) — Ln/Exp share an ACT table set with
    Exp/Copy/Identity, unlike Sqrt (1.28us table reload per switch).
  * all PSUM compute tiles are [128,1024] (2 banks) -> 1024-wide
    exp/copies halve the ACT instruction-overhead (352 cyc/op).
  * stats matmuls (group pool/expand) in bf16 single-pass (no fp32
    LOW_HIGH double-pass); warmups bf16.
  * engine-queue discipline: x loads + out stores on the sync queue,
    denominator bounce on the (otherwise idle) pool queue, no DMA ever
    on the ACT queue mid-pipeline; final example drains stt across
    DVE and ACT+pool in parallel.
  * b_res == 0 fast path: skips the 32 pool residual-bias adds.
"""

import numpy as np

import concourse.bass as bass
import concourse.mybir as mybir
import concourse.tile as tile
from concourse import bacc
from concourse.bass_utils import run_bass_kernel_spmd
from concourse.masks import make_identity

B, H, W, C = 32, 32, 32, 512
N = H * W                      # 1024 positions
G = 32                         # groups
GS = C // G                    # 16 channels per group
EPS = 1e-5
NCORES = 8
BPC = B // NCORES              # 4 examples per core
ISQ = float(1.0 / np.sqrt(C))  # score scale (folded into M' on host)

F32 = mybir.dt.float32
BF16 = mybir.dt.bfloat16
FP8 = mybir.dt.float8e4
AF = mybir.ActivationFunctionType
ALU = mybir.AluOpType
MS = bass.MemorySpace
DR = mybir.MatmulPerfMode.DoubleRow

SCALE_M = 256.0                # host upscale on M'; exp scale undoes it
SCALE_V = 16.0                 # host upscale on Wv' for fp8 range; folded
                               # into the softmax denominator via ones2
EXP_B = -2.0                   # exp range-compression bias (cancels in
                               # softmax)


class Ctx:
    pass


def _load_x(g, bi, qs):
    xn = g.xn_p.tile([128, 8, 512], F32, tag="xn", name=f"xn{bi}")
    for d in range(8):
        qs[d % len(qs)].dma_start(xn[:, d, :], g.xr[bi, :, d, :])
    return xn


def _tr_group(g, bi, xb, xt, st6, t):
    """Transpose one channel-chunk t: 8 bf16 PE transposes (single-pass,
    host-precast x) into one PSUM bank, one 1024-wide copy out, two
    bn_stats."""
    nc = g.nc
    trps = g.pm.tile([128, 1024], F32, tag="pm", name=f"trp{bi}_{t}")
    for i in range(8):
        nc.tensor.matmul(
            trps[:, i * 128:(i + 1) * 128],
            xb[:, i, t * 128:(t + 1) * 128],
            g.identf,
            is_transpose=True,
            start=(i % 4 == 0),
            stop=(i % 4 == 3),
        )
    nc.vector.tensor_copy(xt[:, t, :], trps)
    for hh in range(2):
        nc.vector.bn_stats(st6[:, t, hh, :],
                           xt[:, t, hh * 512:(hh + 1) * 512])


def _stats_a(g, bi, st6):
    """aggr -> m2 -> group-pool matmul -> variance (DVE/PE only; the ACT
    part is split off so it never head-blocks the exp stream)."""
    nc = g.nc
    mv = g.small.tile([128, 4, 2], F32, tag="mv", name=f"mv{bi}")
    for t in range(4):
        nc.vector.bn_aggr(mv[:, t, :], st6[:, t, :, :])
    m2 = g.small.tile([128, 4, 2], BF16, tag="m2", name=f"m2{bi}")
    mm = g.small.tile([128, 4, 1], F32, tag="mm", name=f"mm{bi}")
    nc.vector.tensor_copy(m2[:, :, 0:1], mv[:, :, 0:1])
    nc.vector.tensor_mul(mm, mv[:, :, 0:1], mv[:, :, 0:1])
    nc.vector.tensor_add(m2[:, :, 1:2], mm, mv[:, :, 1:2])
    ps_g = g.aux.tile([8, 4, 2], F32, tag="aux", name=f"ps_g{bi}")
    nc.tensor.matmul(ps_g, g.a_pool, m2, start=True, stop=True)
    pg = g.small.tile([8, 4, 2], F32, tag="pg", name=f"pg{bi}")
    nc.vector.tensor_copy(pg, ps_g)
    vr = g.small.tile([8, 4, 1], F32, tag="vr", name=f"vr{bi}")
    nc.vector.tensor_mul(vr, pg[:, :, 0:1], pg[:, :, 0:1])
    nc.vector.tensor_sub(vr, pg[:, :, 1:2], vr)
    return pg, vr


def _stats_rstd(g, bi, pg, vr):
    """rstd = exp(-0.5*ln(var+eps)) — Ln/Exp live in the same ACT table
    set as Exp/Copy/Identity, unlike Sqrt (1.28us reload per switch)."""
    nc = g.nc
    gab = g.small.tile([8, 4, 2], BF16, tag="gab", name=f"gab{bi}")
    nc.scalar.activation(vr, vr, AF.Ln, bias=g.eps_c[:8])
    nc.scalar.activation(gab[:, :, 0:1], vr, AF.Exp, scale=-0.5)
    nc.vector.tensor_copy(gab[:, :, 1:2], pg[:, :, 0:1])
    return gab


def _stats_b(g, bi, gab):
    """Expand group stats to channels + affine coefficients."""
    nc = g.nc
    ps_ab = g.aux.tile([128, 4, 2], F32, tag="aux", name=f"ps_ab{bi}")
    nc.tensor.matmul(ps_ab, g.e8, gab, start=True, stop=True)
    ab = g.small.tile([128, 4, 2], F32, tag="ab", name=f"ab{bi}")
    tmpc = g.small.tile([128, 4, 1], F32, tag="tmpc", name=f"tmpc{bi}")
    nc.vector.tensor_mul(ab[:, :, 0:1], ps_ab[:, :, 0:1], g.gns_sb[:, :, 0:1])
    nc.vector.tensor_mul(tmpc, ps_ab[:, :, 1:2], ab[:, :, 0:1])
    nc.vector.tensor_sub(ab[:, :, 1:2], g.gnb_sb[:, :, 0:1], tmpc)
    return ab


def _zt16(g, bi, xt, ab):
    # all on DVE: the ST window's ACT is saturated by exp, DVE is idle
    zt16 = g.zt16_p.tile([128, 4, 1024], BF16, tag="zt16", name=f"zt16_{bi}")
    nc = g.nc
    for t in range(4):
        nc.vector.tensor_scalar(
            out=zt16[:, t, :], in0=xt[:, t, :],
            scalar1=ab[:, t, 0:1], scalar2=ab[:, t, 1:2],
            op0=ALU.mult, op1=ALU.add,
        )
    return zt16


def _zt8_part(g, bi, zt16, zt8, ts, eng=None):
    if zt8 is None:
        zt8 = g.zt8_p.tile([128, 4, 1024], FP8, tag="zt8", name=f"zt8_{bi}")
    for t in ts:
        if eng == "act":
            g.nc.scalar.copy(zt8[:, t, :], zt16[:, t, :])
        else:
            g.nc.vector.tensor_copy(zt8[:, t, :], zt16[:, t, :])
    return zt8


def _g_phase(g, bi, zt16, tr=None):
    """G^T = M'^T Z^T in bf16; interleave next example's transposes."""
    nc = g.nc
    gt = g.gt_p.tile([128, 4, 1024], FP8, tag="gt", name=f"gt{bi}")
    for m in range(4):
        ps = g.pm.tile([128, 1024], F32, tag="pm", name=f"ps_g{bi}_{m}")
        for kk in range(4):
            for h in range(2):
                nc.tensor.matmul(
                    ps[:, h * 512:(h + 1) * 512],
                    g.mq_sb[:, kk, m * 128:(m + 1) * 128],
                    zt16[:, kk, h * 512:(h + 1) * 512],
                    start=(kk == 0),
                    stop=(kk == 3),
                )
        nc.scalar.copy(gt[:, m, :], ps)
        if tr is not None:
            tr(m)
    return gt


def _u_stage(g, bi, zt16):
    """Per-key bias u_j = uvec . z_j  (only when b_qkv != 0)."""
    nc = g.nc
    ps_u = g.aux.tile([128, 8], F32, tag="aux", name=f"ps_u{bi}")
    for j in range(8):
        for kk in range(4):
            nc.tensor.matmul(
                ps_u[:, j:j + 1],
                zt16[:, kk, j * 128:(j + 1) * 128],
                g.uv_sb[:, kk:kk + 1],
                start=(kk == 0),
                stop=(kk == 3),
            )
    u_sb = g.small.tile([128, 8], F32, tag="u_sb", name=f"u_sb{bi}")
    nc.vector.tensor_scalar(out=u_sb, in0=ps_u, scalar1=1.0, scalar2=EXP_B,
                            op0=ALU.mult, op1=ALU.add)
    return u_sb


def _st_phase(g, bi, zt8, gt, u_sb=None, hooks=None):
    """S^T + exp -> ET via fp8 DoubleRow; denominators via M=1 DR matmuls;
    hooks emit the next example's stats/zt work mid-loop."""
    nc = g.nc
    hooks = hooks or {}
    et = g.et_p.tile([128, 8, 1024], FP8, tag="et", name=f"et{bi}")
    s_ps = g.sden.tile([1, 2, 512], F32, tag="sden", name=f"sps{bi}")
    for j in range(8):
        ps = g.pm.tile([128, 1024], F32, tag="pm", name=f"ps_s{bi}_{j}")
        for k2 in range(2):
            for h in range(2):
                nc.tensor.matmul(
                    ps[:, h * 512:(h + 1) * 512],
                    zt8[:, 2 * k2:2 * k2 + 2, j * 128:(j + 1) * 128],
                    gt[:, 2 * k2:2 * k2 + 2, h * 512:(h + 1) * 512],
                    start=(k2 == 0),
                    stop=(k2 == 1),
                    perf_mode=DR,
                )
        nc.scalar.activation(
            et[:, j, :], ps, AF.Exp,
            scale=1.0 / SCALE_M,
            bias=g.neg2 if u_sb is None else u_sb[:, j:j + 1])
        # denominator for pair jj emitted two j's after its exps complete,
        # so these matmuls never head-block the PE FIFO on the ACT
        if j in (3, 5, 7):
            jj = (j - 3) // 2
            for h in range(2):
                nc.tensor.matmul(
                    s_ps[:, h, :],
                    g.ones2[:, :, 0:1],
                    et[:, 2 * jj:2 * jj + 2, h * 512:(h + 1) * 512],
                    start=(jj == 0),
                    stop=False,
                    perf_mode=DR,
                )
        if j in hooks:
            hooks[j]()
    return et, s_ps


def _v_phase(g, bi, zt8, after_p0=None):
    """V' = Z Wv' via fp8 DoubleRow; 512-wide fp8 copies alternating
    DVE/ACT so the last copy lags the last fill minimally."""
    nc = g.nc
    v = g.v_p.tile([128, 8, 512], FP8, tag="v", name=f"v{bi}")
    for p in range(4):
        ps = g.pm.tile([128, 1024], F32, tag="pm", name=f"ps_v{bi}_{p}")
        for k2 in range(2):
            for io in range(2):
                i = 2 * p + io
                nc.tensor.matmul(
                    ps[:, io * 512:(io + 1) * 512],
                    zt8[:, 2 * k2:2 * k2 + 2, i * 128:(i + 1) * 128],
                    g.wvp_sb[:, 2 * k2:2 * k2 + 2, :],
                    start=(k2 == 0),
                    stop=(k2 == 1),
                    perf_mode=DR,
                )
        for io in range(2):
            i = 2 * p + io
            if io == 0:
                nc.vector.tensor_copy(v[:, i, :], ps[:, io * 512:(io + 1) * 512])
            else:
                nc.scalar.copy(v[:, i, :], ps[:, io * 512:(io + 1) * 512])
        if p == 0 and after_p0 is not None:
            after_p0()
    return v


def _denom_last(g, bi, s_ps, et):
    """Final denominator pair (jj=3) plus the row->column DRAM bounce,
    emitted inside the V phase so rc is back before the O phase needs
    it."""
    nc = g.nc
    for h in range(2):
        nc.tensor.matmul(
            s_ps[:, h, :],
            g.ones2[:, :, 0:1],
            et[:, 6:8, h * 512:(h + 1) * 512],
            start=False,
            stop=True,
            perf_mode=DR,
        )
    s_sb = g.small.tile([1, 1024], F32, tag="s_sb", name=f"s_sb{bi}")
    for h in range(2):
        nc.vector.tensor_copy(s_sb[:, h * 512:(h + 1) * 512], s_ps[:, h, :])
    s_dram = g.dram.tile([1, 1024], F32, tag="s_dram", name=f"s_dram{bi}")
    nc.gpsimd.dma_start(s_dram, s_sb)
    s_col = g.small.tile([128, 8], F32, tag="s_col", name=f"s_col{bi}")
    nc.gpsimd.dma_start(s_col, s_dram.rearrange("o (t p) -> p (o t)", p=128))
    return s_col


def _denom_fin(g, bi, s_col):
    rc = g.small.tile([128, 8], F32, tag="rc", name=f"rc{bi}")
    g.nc.vector.reciprocal(rc, s_col)
    return rc


def _o_phase(g, bi, xn, v, et, rc, has_b, last=False):
    """O natural via fp8 DoubleRow, residual stt, store."""
    nc = g.nc
    res = g.res_p.tile([128, 8, 512], F32, tag="res", name=f"res{bi}")
    out_q = [nc.sync] if not last else [nc.sync, nc.scalar, nc.gpsimd]
    if has_b:
        for i in range(8):
            nc.gpsimd.tensor_add(xn[:, i, :], xn[:, i, :], g.bres_bc)
    for p in range(4):
        ps = g.pm.tile([128, 1024], F32, tag="pm", name=f"ps_o{bi}_{p}")
        for io in range(2):
            i = 2 * p + io
            for jj in range(4):
                nc.tensor.matmul(
                    ps[:, io * 512:(io + 1) * 512],
                    et[:, 2 * jj:2 * jj + 2, i * 128:(i + 1) * 128],
                    v[:, 2 * jj:2 * jj + 2, :],
                    start=(jj == 0),
                    stop=(jj == 3),
                    perf_mode=DR,
                )
        for io in range(2):
            i = 2 * p + io
            if last and io == 1:
                # final-example drain, 3-way: ACT scales; the residual
                # add goes to the pool for the early chunks and to the
                # DVE for the late ones (a pool add is 1.27us vs 0.74
                # on DVE — the serial pool chain was the old tail)
                nc.scalar.activation(res[:, i, :],
                                     ps[:, io * 512:(io + 1) * 512],
                                     AF.Identity, scale=rc[:, i:i + 1])
                if p < 2:
                    nc.gpsimd.tensor_add(res[:, i, :], res[:, i, :],
                                         xn[:, i, :])
                else:
                    nc.vector.tensor_add(res[:, i, :], res[:, i, :],
                                         xn[:, i, :])
            else:
                nc.vector.scalar_tensor_tensor(
                    out=res[:, i, :], in0=ps[:, io * 512:(io + 1) * 512],
                    scalar=rc[:, i:i + 1],
                    in1=xn[:, i, :], op0=ALU.mult, op1=ALU.add,
                )
            out_q[i % len(out_q)].dma_start(g.outr[bi, :, i, :], res[:, i, :])


def build_program(has_u, has_b):
    nc = bacc.Bacc("TRN2", target_bir_lowering=False, debug=False)

    x_d = nc.dram_tensor("x", [BPC, N, C], F32, kind="ExternalInput")
    mq_d = nc.dram_tensor("m_qk", [C, C], BF16, kind="ExternalInput")
    wvp_d = nc.dram_tensor("w_vp", [C, C], FP8, kind="ExternalInput")
    gns_d = nc.dram_tensor("gn_scale", [C], F32, kind="ExternalInput")
    gnb_d = nc.dram_tensor("gn_bias", [C], F32, kind="ExternalInput")
    if has_b:
        bres_d = nc.dram_tensor("b_res", [C], F32, kind="ExternalInput")
    if has_u:
        uv_d = nc.dram_tensor("u_vec", [C], F32, kind="ExternalInput")
    out_d = nc.dram_tensor("out", [BPC, N, C], F32, kind="ExternalOutput")

    g = Ctx()
    g.nc = nc
    g.xr = x_d.ap().rearrange("b (i p) c -> b p i c", p=128)
    g.outr = out_d.ap().rearrange("b (i p) c -> b p i c", p=128)

    with tile.TileContext(nc) as tc:
        from contextlib import ExitStack
        with ExitStack() as ctx:
            const = ctx.enter_context(tc.tile_pool(name="const", bufs=1))
            g.pm = ctx.enter_context(tc.tile_pool(name="pm", bufs=2, space=MS.PSUM))
            g.sden = ctx.enter_context(tc.tile_pool(name="sden", bufs=1, space=MS.PSUM))
            g.aux = ctx.enter_context(tc.tile_pool(name="aux", bufs=2, space=MS.PSUM))
            g.xn_p = ctx.enter_context(tc.tile_pool(name="xn", bufs=3))
            g.xt_p = ctx.enter_context(tc.tile_pool(name="xtp", bufs=2))
            g.zt16_p = ctx.enter_context(tc.tile_pool(name="zt16p", bufs=2))
            g.zt8_p = ctx.enter_context(tc.tile_pool(name="zt8p", bufs=2))
            g.gt_p = ctx.enter_context(tc.tile_pool(name="gtp", bufs=2))
            g.v_p = ctx.enter_context(tc.tile_pool(name="vp", bufs=2))
            g.et_p = ctx.enter_context(tc.tile_pool(name="etp", bufs=2))
            g.res_p = ctx.enter_context(tc.tile_pool(name="resp", bufs=1))
            g.small = ctx.enter_context(tc.tile_pool(name="small", bufs=3))
            g.dram = ctx.enter_context(tc.tile_pool(name="dram", bufs=2,
                                                    space=MS.DRAM))

            # ---- input DMA for examples 0 and 1 first: critical path ----
            xn0 = _load_x(g, 0, [nc.sync, nc.scalar, nc.gpsimd])
            xn1 = _load_x(g, 1, [nc.gpsimd, nc.sync, nc.scalar])

            # ---- constants ----------------------------------------------
            g.ident = const.tile([128, 128], BF16)
            make_identity(nc, g.ident)
            g.identf = const.tile([128, 128], F32)
            make_identity(nc, g.identf)
            g.junk = const.tile([128, 512], BF16)
            nc.vector.memset(g.junk, 0.5)

            # PE warmup: bf16 matmuls with no DMA dependency, issued while
            # the input DMAs run, so the HAM clock gate reaches K=8/8
            # before the first productive matmul.
            def warm(n, salt=[0]):
                for _ in range(n):
                    salt[0] += 1
                    ps_w = g.pm.tile([128, 1024], F32, tag="pm",
                                     name=f"ps_w{salt[0]}")
                    nc.tensor.matmul(ps_w[:, 0:512], g.ident, g.junk,
                                     start=True, stop=True)
            g.warm = warm
            warm(10)

            g.a_pool = const.tile([128, 8], BF16)
            nc.gpsimd.memset(g.a_pool, 1.0 / GS)
            nc.gpsimd.affine_select(
                out=g.a_pool, in_=g.a_pool, compare_op=ALU.is_ge, fill=0.0,
                base=0, pattern=[[-GS, 8]], channel_multiplier=1)
            nc.gpsimd.affine_select(
                out=g.a_pool, in_=g.a_pool, compare_op=ALU.is_ge, fill=0.0,
                base=GS - 1, pattern=[[GS, 8]], channel_multiplier=-1)

            g.e8 = const.tile([8, 128], BF16)
            nc.gpsimd.memset(g.e8, 1.0)
            nc.gpsimd.affine_select(
                out=g.e8, in_=g.e8, compare_op=ALU.is_ge, fill=0.0,
                base=0, pattern=[[1, 128]], channel_multiplier=-GS)
            nc.gpsimd.affine_select(
                out=g.e8, in_=g.e8, compare_op=ALU.is_ge, fill=0.0,
                base=GS - 1, pattern=[[-1, 128]], channel_multiplier=GS)

            g.ones2 = const.tile([128, 2, 16], FP8)
            nc.vector.memset(g.ones2, SCALE_V)
            g.neg2 = const.tile([128, 1], F32)
            nc.vector.memset(g.neg2, EXP_B)
            g.eps_c = const.tile([128, 1], F32)
            nc.vector.memset(g.eps_c, EPS)

            g.mq_sb = const.tile([128, 4, C], BF16)
            mqr = mq_d.ap().rearrange("(t p) d -> t p d", p=128)
            for t in range(4):
                nc.gpsimd.dma_start(g.mq_sb[:, t, :], mqr[t])
            g.wvp_sb = const.tile([128, 4, C], FP8)
            wvr = wvp_d.ap().rearrange("(t p) d -> t p d", p=128)
            for t in range(4):
                nc.gpsimd.dma_start(g.wvp_sb[:, t, :], wvr[t])

            g.gns_sb = const.tile([128, 4, 1], F32)
            nc.sync.dma_start(g.gns_sb[:, :, 0:1],
                              gns_d.ap().rearrange("(t p) -> p t", p=128))
            g.gnb_sb = const.tile([128, 4, 1], F32)
            nc.sync.dma_start(g.gnb_sb[:, :, 0:1],
                              gnb_d.ap().rearrange("(t p) -> p t", p=128))

            def bcast(src_ap):
                return bass.AP(
                    tensor=src_ap.tensor, offset=src_ap.offset,
                    ap=[[0, 128]] + [list(p) for p in src_ap.ap])

            if has_b:
                g.bres_bc = const.tile([128, 512], F32)
                nc.gpsimd.dma_start(g.bres_bc, bcast(bres_d.ap()))

            if has_u:
                uvf = const.tile([128, 4], F32)
                nc.sync.dma_start(uvf, uv_d.ap().rearrange("(t p) -> p t", p=128))
                g.uv_sb = const.tile([128, 4], BF16)
                nc.vector.tensor_copy(g.uv_sb, uvf)

            # ---- example-0 prologue -------------------------------------
            warm(40)
            xt0 = g.xt_p.tile([128, 4, 1024], BF16, tag="xt", name="xt0")
            st6_0 = g.small.tile([128, 4, 2, 6], F32, tag="st6", name="st6_0")
            for t in range(4):
                _tr_group(g, 0, xn0, xt0, st6_0, t)
            warm(8)
            pg0, vr0 = _stats_a(g, 0, st6_0)
            gab0 = _stats_rstd(g, 0, pg0, vr0)
            warm(14)
            ab0 = _stats_b(g, 0, gab0)
            warm(10)
            zt16_0 = _zt16(g, 0, xt0, ab0)
            warm(10)
            zt8_0 = _zt8_part(g, 0, zt16_0, None, (0, 1, 2, 3))
            warm(6)

            # ---- pipelined per-example emission -------------------------
            xns = {0: xn0, 1: xn1}
            state = (zt16_0, zt8_0)
            nxt = {}
            for bi in range(BPC):
                zt16, zt8 = state
                xn = xns[bi]
                if bi + 2 < BPC:
                    xns[bi + 2] = _load_x(g, bi + 2, [nc.sync])
                tr = None
                hooks = {}
                if bi + 1 < BPC:
                    xb1_t = xns[bi + 1]
                    xt1 = g.xt_p.tile([128, 4, 1024], BF16, tag="xt",
                                      name=f"xt{bi+1}")
                    st6 = g.small.tile([128, 4, 2, 6], F32, tag="st6",
                                       name=f"st6_{bi+1}")

                    def tr(m, xb1_t=xb1_t, xt1=xt1, st6=st6, b1=bi + 1):
                        _tr_group(g, b1, xb1_t, xt1, st6, m)
                gt = _g_phase(g, bi, zt16, tr)
                if bi + 1 < BPC:
                    nxt["pgvr"] = _stats_a(g, bi + 1, st6)

                    def hook2(b1=bi + 1):
                        nxt["gab"] = _stats_rstd(g, b1, *nxt["pgvr"])

                    def hook4(b1=bi + 1):
                        nxt["ab"] = _stats_b(g, b1, nxt["gab"])

                    def hook6(xt1=xt1, b1=bi + 1):
                        nxt["zt16"] = _zt16(g, b1, xt1, nxt["ab"])
                    hooks = {2: hook2, 4: hook4, 6: hook6}
                u_sb = _u_stage(g, bi, zt16) if has_u else None
                et, s_ps = _st_phase(g, bi, zt8, gt, u_sb, hooks)
                box = {}

                def after_p0(box=box, s_ps=s_ps, et=et, b=bi):
                    box["s_col"] = _denom_last(g, b, s_ps, et)
                v = _v_phase(g, bi, zt8, after_p0=after_p0)
                rc = _denom_fin(g, bi, box["s_col"])
                if bi + 1 < BPC:
                    nxt["zt8"] = _zt8_part(g, bi + 1, nxt["zt16"],
                                           None, (0, 1, 2, 3), eng="act")
                _o_phase(g, bi, xn, v, et, rc, has_b, last=(bi == BPC - 1))
                if bi + 1 < BPC:
                    state = (nxt["zt16"], nxt["zt8"])

    nc.compile()
    return nc


_NC = {}


def _get_nc(has_u, has_b):
    key = (has_u, has_b)
    if key not in _NC:
        _NC[key] = build_program(has_u, has_b)
    return _NC[key]


def kernel(x, t, gn_scale, gn_bias, w_qkv, b_qkv, w_out, b_out):
    import ml_dtypes
    x = np.ascontiguousarray(np.asarray(x, np.float32).reshape(B, N, C))
    w_qkv = np.asarray(w_qkv, np.float32)
    b_qkv = np.asarray(b_qkv, np.float32)
    w_out = np.asarray(w_out, np.float32)
    b_out = np.asarray(b_out, np.float32)
    wq, wk, wv = w_qkv[:, 0:C], w_qkv[:, C:2 * C], w_qkv[:, 2 * C:3 * C]
    bq, bv = b_qkv[0:C], b_qkv[2 * C:3 * C]

    m_qk = ((wq @ wk.T) * (ISQ * SCALE_M)).astype(ml_dtypes.bfloat16)
    w_vp = np.clip((wv @ w_out) * SCALE_V, -240.0, 240.0).astype(
        ml_dtypes.float8_e4m3)
    b_res = (bv @ w_out + b_out).astype(np.float32)
    u_vec = ((wk @ bq) * ISQ).astype(np.float32)
    has_u = bool(np.any(u_vec != 0.0))
    has_b = bool(np.any(b_res != 0.0))

    shared = {
        "m_qk": np.ascontiguousarray(m_qk),
        "w_vp": np.ascontiguousarray(w_vp),
        "gn_scale": np.ascontiguousarray(np.asarray(gn_scale, np.float32)),
        "gn_bias": np.ascontiguousarray(np.asarray(gn_bias, np.float32)),
    }
    if has_b:
        shared["b_res"] = np.ascontiguousarray(b_res)
    if has_u:
        shared["u_vec"] = np.ascontiguousarray(u_vec)
    in_maps = [
        {"x": x[c * BPC:(c + 1) * BPC], **shared} for c in range(NCORES)
    ]
    nc = _get_nc(has_u, has_b)
    res = run_bass_kernel_spmd(nc, in_maps, core_ids=list(range(NCORES)))
    out = np.concatenate([res.results[c]["out"] for c in range(NCORES)], axis=0)
    return out.reshape(B, H, W, C)


# revision 68
# speedup vs baseline: 1.0277x; 1.0272x over previous
"""TRN2 Bass kernel for nn_Attention_20444044329649.

GroupNorm(32) -> qkv dense -> single-head spatial attention (1024 pos) ->
out dense -> residual.  B=32 examples sharded 4-per-core across 8 cores;
params replicated.

v4 — fp8 DoubleRow main pipeline + phase-shifted stats (vs v3: 249us ->
~209us):

  * scores:  S*isq = Z M' Z^T with M' = isq*Wq Wk^T host-precomputed (kept
    bf16 — fp8 M' was measured as the dominant systematic error source).
    G^T = M'^T Z^T in bf16; S^T = Z_j M'^T Z^T via fp8 DoubleRow
    (zt8 x gt8) — half the matmul instructions of the bf16 path.
  * V' = Z Wv' via fp8 DoubleRow (zt8 x wvp8, host-folded Wv*W_out*16;
    the *16 restores fp8 range, undone via the ones2=16 denominator).
  * O = A V via fp8 DoubleRow; denominators via M=1 ones-lhsT DoubleRow
    matmuls emitted two j's after their exps (never head-block the PE
    FIFO on the ACT), last pair inside the V phase together with the
    row->column DRAM bounce so rc lands before the O phase.
  * phase-shifted stats: transposes for bi+1 interleave into G-phase(bi)
    (PSUM tiles from the shared pm pool); bn_stats/aggr/pool chain runs
    during ST(bi) via hooks (rstd at j==2, affine at j==4, zt16 at
    j==6); zt8 copies on ACT during the O window.  The PE never waits
    on the serial stats chain at example boundaries (v3 lost
    ~6.8us/example to HAM re-throttle there).
  * rstd = exp(-0.5*ln(var+eps)) — Ln/Exp share an ACT table set with
    Exp/Copy/Identity, unlike Sqrt (1.28us table reload per switch).
  * all PSUM compute tiles are [128,1024] (2 banks) -> 1024-wide
    exp/copies halve the ACT instruction-overhead (352 cyc/op).
  * stats matmuls (group pool/expand) in bf16 single-pass (no fp32
    LOW_HIGH double-pass); warmups bf16.
  * engine-queue discipline: x loads + out stores on the sync queue,
    denominator bounce on the (otherwise idle) pool queue, no DMA ever
    on the ACT queue mid-pipeline; final example drains stt across
    DVE and ACT+pool in parallel.
  * b_res == 0 fast path: skips the 32 pool residual-bias adds.
"""

import numpy as np

import concourse.bass as bass
import concourse.mybir as mybir
import concourse.tile as tile
from concourse import bacc
from concourse.bass_utils import run_bass_kernel_spmd
from concourse.masks import make_identity

B, H, W, C = 32, 32, 32, 512
N = H * W                      # 1024 positions
G = 32                         # groups
GS = C // G                    # 16 channels per group
EPS = 1e-5
NCORES = 8
BPC = B // NCORES              # 4 examples per core
ISQ = float(1.0 / np.sqrt(C))  # score scale (folded into M' on host)

F32 = mybir.dt.float32
BF16 = mybir.dt.bfloat16
FP8 = mybir.dt.float8e4
AF = mybir.ActivationFunctionType
ALU = mybir.AluOpType
MS = bass.MemorySpace
DR = mybir.MatmulPerfMode.DoubleRow

SCALE_M = 256.0                # host upscale on M'; exp scale undoes it
SCALE_V = 16.0                 # host upscale on Wv' for fp8 range; folded
                               # into the softmax denominator via ones2
EXP_B = -2.0                   # exp range-compression bias (cancels in
                               # softmax)


class Ctx:
    pass


def _load_x(g, bi, qs):
    xn = g.xn_p.tile([128, 8, 512], F32, tag="xn", name=f"xn{bi}")
    for d in range(8):
        qs[d % len(qs)].dma_start(xn[:, d, :], g.xr[bi, :, d, :])
    return xn


def _tr_group(g, bi, xb, xt, st6, t):
    """Transpose one channel-chunk t: 8 bf16 PE transposes (single-pass,
    host-precast x) into one PSUM bank, one 1024-wide copy out, two
    bn_stats."""
    nc = g.nc
    trps = g.trp.tile([128, 1024], F32, tag="trp", name=f"trp{bi}_{t}")
    for i in range(8):
        nc.tensor.matmul(
            trps[:, i * 128:(i + 1) * 128],
            xb[:, i, t * 128:(t + 1) * 128],
            g.identf,
            is_transpose=True,
            start=(i % 4 == 0),
            stop=(i % 4 == 3),
        )
    nc.vector.tensor_copy(xt[:, t, :], trps)
    for hh in range(2):
        nc.vector.bn_stats(st6[:, t, hh, :],
                           xt[:, t, hh * 512:(hh + 1) * 512])


def _stats_a(g, bi, st6):
    """aggr -> m2 -> group-pool matmul -> variance (DVE/PE only; the ACT
    part is split off so it never head-blocks the exp stream)."""
    nc = g.nc
    mv = g.small.tile([128, 4, 2], F32, tag="mv", name=f"mv{bi}")
    for t in range(4):
        nc.vector.bn_aggr(mv[:, t, :], st6[:, t, :, :])
    m2 = g.small.tile([128, 4, 2], BF16, tag="m2", name=f"m2{bi}")
    mm = g.small.tile([128, 4, 1], F32, tag="mm", name=f"mm{bi}")
    nc.vector.tensor_copy(m2[:, :, 0:1], mv[:, :, 0:1])
    nc.vector.tensor_mul(mm, mv[:, :, 0:1], mv[:, :, 0:1])
    nc.vector.tensor_add(m2[:, :, 1:2], mm, mv[:, :, 1:2])
    ps_g = g.pm.tile([8, 4, 2], F32, tag="pm", name=f"ps_g{bi}")
    nc.tensor.matmul(ps_g, g.a_pool, m2, start=True, stop=True)
    pg = g.small.tile([8, 4, 2], F32, tag="pg", name=f"pg{bi}")
    nc.vector.tensor_copy(pg, ps_g)
    vr = g.small.tile([8, 4, 1], F32, tag="vr", name=f"vr{bi}")
    nc.vector.tensor_mul(vr, pg[:, :, 0:1], pg[:, :, 0:1])
    nc.vector.tensor_sub(vr, pg[:, :, 1:2], vr)
    return pg, vr


def _stats_rstd(g, bi, pg, vr):
    """rstd = exp(-0.5*ln(var+eps)) — Ln/Exp live in the same ACT table
    set as Exp/Copy/Identity, unlike Sqrt (1.28us reload per switch)."""
    nc = g.nc
    gab = g.small.tile([8, 4, 2], BF16, tag="gab", name=f"gab{bi}")
    nc.scalar.activation(vr, vr, AF.Ln, bias=g.eps_c[:8])
    nc.scalar.activation(gab[:, :, 0:1], vr, AF.Exp, scale=-0.5)
    nc.vector.tensor_copy(gab[:, :, 1:2], pg[:, :, 0:1])
    return gab


def _stats_b(g, bi, gab):
    """Expand group stats to channels + affine coefficients."""
    nc = g.nc
    ps_ab = g.pm.tile([128, 4, 2], F32, tag="pm", name=f"ps_ab{bi}")
    nc.tensor.matmul(ps_ab, g.e8, gab, start=True, stop=True)
    ab = g.small.tile([128, 4, 2], F32, tag="ab", name=f"ab{bi}")
    tmpc = g.small.tile([128, 4, 1], F32, tag="tmpc", name=f"tmpc{bi}")
    nc.vector.tensor_mul(ab[:, :, 0:1], ps_ab[:, :, 0:1], g.gns_sb[:, :, 0:1])
    nc.vector.tensor_mul(tmpc, ps_ab[:, :, 1:2], ab[:, :, 0:1])
    nc.vector.tensor_sub(ab[:, :, 1:2], g.gnb_sb[:, :, 0:1], tmpc)
    return ab


def _zt16(g, bi, xt, ab):
    # all on DVE: the ST window's ACT is saturated by exp, DVE is idle
    zt16 = g.zt16_p.tile([128, 4, 1024], BF16, tag="zt16", name=f"zt16_{bi}")
    nc = g.nc
    for t in range(4):
        nc.vector.tensor_scalar(
            out=zt16[:, t, :], in0=xt[:, t, :],
            scalar1=ab[:, t, 0:1], scalar2=ab[:, t, 1:2],
            op0=ALU.mult, op1=ALU.add,
        )
    return zt16


def _zt8_part(g, bi, zt16, zt8, ts, eng=None):
    if zt8 is None:
        zt8 = g.zt8_p.tile([128, 4, 1024], FP8, tag="zt8", name=f"zt8_{bi}")
    for t in ts:
        if eng == "act":
            g.nc.scalar.copy(zt8[:, t, :], zt16[:, t, :])
        else:
            g.nc.vector.tensor_copy(zt8[:, t, :], zt16[:, t, :])
    return zt8


def _g_phase(g, bi, zt16, tr=None):
    """G^T = M'^T Z^T in bf16; interleave next example's transposes."""
    nc = g.nc
    gt = g.gt_p.tile([128, 4, 1024], FP8, tag="gt", name=f"gt{bi}")
    for m in range(4):
        ps = g.pm.tile([128, 1024], F32, tag="pm", name=f"ps_g{bi}_{m}")
        for kk in range(4):
            for h in range(2):
                nc.tensor.matmul(
                    ps[:, h * 512:(h + 1) * 512],
                    g.mq_sb[:, kk, m * 128:(m + 1) * 128],
                    zt16[:, kk, h * 512:(h + 1) * 512],
                    start=(kk == 0),
                    stop=(kk == 3),
                )
        nc.scalar.copy(gt[:, m, :], ps)
        if tr is not None:
            tr(m)
    return gt


def _u_stage(g, bi, zt16):
    """Per-key bias u_j = uvec . z_j  (only when b_qkv != 0)."""
    nc = g.nc
    ps_u = g.pm.tile([128, 8], F32, tag="pm", name=f"ps_u{bi}")
    for j in range(8):
        for kk in range(4):
            nc.tensor.matmul(
                ps_u[:, j:j + 1],
                zt16[:, kk, j * 128:(j + 1) * 128],
                g.uv_sb[:, kk:kk + 1],
                start=(kk == 0),
                stop=(kk == 3),
            )
    u_sb = g.small.tile([128, 8], F32, tag="u_sb", name=f"u_sb{bi}")
    nc.vector.tensor_scalar(out=u_sb, in0=ps_u, scalar1=1.0, scalar2=EXP_B,
                            op0=ALU.mult, op1=ALU.add)
    return u_sb


def _st_phase(g, bi, zt8, gt, u_sb=None, hooks=None):
    """S^T + exp -> ET via fp8 DoubleRow; denominators via M=1 DR matmuls;
    hooks emit the next example's stats/zt work mid-loop."""
    nc = g.nc
    hooks = hooks or {}
    et = g.et_p.tile([128, 8, 1024], FP8, tag="et", name=f"et{bi}")
    s_ps = g.sden.tile([1, 2, 512], F32, tag="sden", name=f"sps{bi}")
    for j in range(8):
        # j=2,5 borrow the transpose pool's banks (idle during ST): a
        # 3-deep rotation keeps each exp-recycle wait under the HAM
        # idle threshold instead of one 2.7us lump
        pool, tg = (g.trp, "trp") if j in (2, 5) else (g.pm, "pm")
        ps = pool.tile([128, 1024], F32, tag=tg, name=f"ps_s{bi}_{j}")
        for k2 in range(2):
            for h in range(2):
                nc.tensor.matmul(
                    ps[:, h * 512:(h + 1) * 512],
                    zt8[:, 2 * k2:2 * k2 + 2, j * 128:(j + 1) * 128],
                    gt[:, 2 * k2:2 * k2 + 2, h * 512:(h + 1) * 512],
                    start=(k2 == 0),
                    stop=(k2 == 1),
                    perf_mode=DR,
                )
        nc.scalar.activation(
            et[:, j, :], ps, AF.Exp,
            scale=1.0 / SCALE_M,
            bias=g.neg2 if u_sb is None else u_sb[:, j:j + 1])
        # denominator for pair jj emitted two j's after its exps complete,
        # so these matmuls never head-block the PE FIFO on the ACT
        if j in (3, 5, 7):
            jj = (j - 3) // 2
            for h in range(2):
                nc.tensor.matmul(
                    s_ps[:, h, :],
                    g.ones2[:, :, 0:1],
                    et[:, 2 * jj:2 * jj + 2, h * 512:(h + 1) * 512],
                    start=(jj == 0),
                    stop=False,
                    perf_mode=DR,
                )
        if j in hooks:
            hooks[j]()
    return et, s_ps


def _v_phase(g, bi, zt8, after_p0=None):
    """V' = Z Wv' via fp8 DoubleRow; 512-wide fp8 copies alternating
    DVE/ACT so the last copy lags the last fill minimally."""
    nc = g.nc
    v = g.v_p.tile([128, 8, 512], FP8, tag="v", name=f"v{bi}")
    for p in range(4):
        ps = g.pm.tile([128, 1024], F32, tag="pm", name=f"ps_v{bi}_{p}")
        for k2 in range(2):
            for io in range(2):
                i = 2 * p + io
                nc.tensor.matmul(
                    ps[:, io * 512:(io + 1) * 512],
                    zt8[:, 2 * k2:2 * k2 + 2, i * 128:(i + 1) * 128],
                    g.wvp_sb[:, 2 * k2:2 * k2 + 2, :],
                    start=(k2 == 0),
                    stop=(k2 == 1),
                    perf_mode=DR,
                )
        for io in range(2):
            i = 2 * p + io
            if io == 0:
                nc.vector.tensor_copy(v[:, i, :], ps[:, io * 512:(io + 1) * 512])
            else:
                nc.scalar.copy(v[:, i, :], ps[:, io * 512:(io + 1) * 512])
        if p == 0 and after_p0 is not None:
            after_p0()
    return v


def _denom_last(g, bi, s_ps, et):
    """Final denominator pair (jj=3) plus the row->column DRAM bounce,
    emitted inside the V phase so rc is back before the O phase needs
    it."""
    nc = g.nc
    for h in range(2):
        nc.tensor.matmul(
            s_ps[:, h, :],
            g.ones2[:, :, 0:1],
            et[:, 6:8, h * 512:(h + 1) * 512],
            start=False,
            stop=True,
            perf_mode=DR,
        )
    s_sb = g.small.tile([1, 1024], F32, tag="s_sb", name=f"s_sb{bi}")
    for h in range(2):
        nc.vector.tensor_copy(s_sb[:, h * 512:(h + 1) * 512], s_ps[:, h, :])
    s_dram = g.dram.tile([1, 1024], F32, tag="s_dram", name=f"s_dram{bi}")
    nc.gpsimd.dma_start(s_dram, s_sb)
    s_col = g.small.tile([128, 8], F32, tag="s_col", name=f"s_col{bi}")
    nc.gpsimd.dma_start(s_col, s_dram.rearrange("o (t p) -> p (o t)", p=128))
    return s_col


def _denom_fin(g, bi, s_col):
    rc = g.small.tile([128, 8], F32, tag="rc", name=f"rc{bi}")
    g.nc.vector.reciprocal(rc, s_col)
    return rc


def _o_phase(g, bi, xn, v, et, rc, has_b, last=False):
    """O natural via fp8 DoubleRow, residual stt, store."""
    nc = g.nc
    res = g.res_p.tile([128, 8, 512], F32, tag="res", name=f"res{bi}")
    out_q = [nc.sync] if not last else [nc.sync, nc.scalar, nc.gpsimd]
    if has_b:
        for i in range(8):
            nc.gpsimd.tensor_add(xn[:, i, :], xn[:, i, :], g.bres_bc)
    for p in range(4):
        ps = g.pm.tile([128, 1024], F32, tag="pm", name=f"ps_o{bi}_{p}")
        for io in range(2):
            i = 2 * p + io
            for jj in range(4):
                nc.tensor.matmul(
                    ps[:, io * 512:(io + 1) * 512],
                    et[:, 2 * jj:2 * jj + 2, i * 128:(i + 1) * 128],
                    v[:, 2 * jj:2 * jj + 2, :],
                    start=(jj == 0),
                    stop=(jj == 3),
                    perf_mode=DR,
                )
        for io in range(2):
            i = 2 * p + io
            if last and io == 1:
                # final-example drain, 3-way: ACT scales; the residual
                # add goes to the pool for the early chunks and to the
                # DVE for the late ones (a pool add is 1.27us vs 0.74
                # on DVE — the serial pool chain was the old tail)
                nc.scalar.activation(res[:, i, :],
                                     ps[:, io * 512:(io + 1) * 512],
                                     AF.Identity, scale=rc[:, i:i + 1])
                if p < 2:
                    nc.gpsimd.tensor_add(res[:, i, :], res[:, i, :],
                                         xn[:, i, :])
                else:
                    nc.vector.tensor_add(res[:, i, :], res[:, i, :],
                                         xn[:, i, :])
            else:
                nc.vector.scalar_tensor_tensor(
                    out=res[:, i, :], in0=ps[:, io * 512:(io + 1) * 512],
                    scalar=rc[:, i:i + 1],
                    in1=xn[:, i, :], op0=ALU.mult, op1=ALU.add,
                )
            out_q[i % len(out_q)].dma_start(g.outr[bi, :, i, :], res[:, i, :])


def build_program(has_u, has_b):
    nc = bacc.Bacc("TRN2", target_bir_lowering=False, debug=False)

    x_d = nc.dram_tensor("x", [BPC, N, C], F32, kind="ExternalInput")
    mq_d = nc.dram_tensor("m_qk", [C, C], BF16, kind="ExternalInput")
    wvp_d = nc.dram_tensor("w_vp", [C, C], FP8, kind="ExternalInput")
    gns_d = nc.dram_tensor("gn_scale", [C], F32, kind="ExternalInput")
    gnb_d = nc.dram_tensor("gn_bias", [C], F32, kind="ExternalInput")
    if has_b:
        bres_d = nc.dram_tensor("b_res", [C], F32, kind="ExternalInput")
    if has_u:
        uv_d = nc.dram_tensor("u_vec", [C], F32, kind="ExternalInput")
    out_d = nc.dram_tensor("out", [BPC, N, C], F32, kind="ExternalOutput")

    g = Ctx()
    g.nc = nc
    g.xr = x_d.ap().rearrange("b (i p) c -> b p i c", p=128)
    g.outr = out_d.ap().rearrange("b (i p) c -> b p i c", p=128)

    with tile.TileContext(nc) as tc:
        from contextlib import ExitStack
        with ExitStack() as ctx:
            const = ctx.enter_context(tc.tile_pool(name="const", bufs=1))
            g.pm = ctx.enter_context(tc.tile_pool(name="pm", bufs=2, space=MS.PSUM))
            g.sden = ctx.enter_context(tc.tile_pool(name="sden", bufs=1, space=MS.PSUM))
            g.trp = ctx.enter_context(tc.tile_pool(name="trp", bufs=1, space=MS.PSUM))
            g.xn_p = ctx.enter_context(tc.tile_pool(name="xn", bufs=3))
            g.xt_p = ctx.enter_context(tc.tile_pool(name="xtp", bufs=2))
            g.zt16_p = ctx.enter_context(tc.tile_pool(name="zt16p", bufs=2))
            g.zt8_p = ctx.enter_context(tc.tile_pool(name="zt8p", bufs=2))
            g.gt_p = ctx.enter_context(tc.tile_pool(name="gtp", bufs=2))
            g.v_p = ctx.enter_context(tc.tile_pool(name="vp", bufs=2))
            g.et_p = ctx.enter_context(tc.tile_pool(name="etp", bufs=2))
            g.res_p = ctx.enter_context(tc.tile_pool(name="resp", bufs=1))
            g.small = ctx.enter_context(tc.tile_pool(name="small", bufs=3))
            g.dram = ctx.enter_context(tc.tile_pool(name="dram", bufs=2,
                                                    space=MS.DRAM))

            # ---- input DMA for examples 0 and 1 first: critical path ----
            xn0 = _load_x(g, 0, [nc.sync, nc.scalar, nc.gpsimd])
            xn1 = _load_x(g, 1, [nc.gpsimd, nc.sync, nc.scalar])

            # ---- constants ----------------------------------------------
            g.ident = const.tile([128, 128], BF16)
            make_identity(nc, g.ident)
            g.identf = const.tile([128, 128], F32)
            make_identity(nc, g.identf)
            g.junk = const.tile([128, 512], BF16)
            nc.vector.memset(g.junk, 0.5)

            # PE warmup: bf16 matmuls with no DMA dependency, issued while
            # the input DMAs run, so the HAM clock gate reaches K=8/8
            # before the first productive matmul.
            def warm(n, salt=[0]):
                for _ in range(n):
                    salt[0] += 1
                    ps_w = g.pm.tile([128, 1024], F32, tag="pm",
                                     name=f"ps_w{salt[0]}")
                    nc.tensor.matmul(ps_w[:, 0:512], g.ident, g.junk,
                                     start=True, stop=True)
            g.warm = warm
            warm(10)

            g.a_pool = const.tile([128, 8], BF16)
            nc.gpsimd.memset(g.a_pool, 1.0 / GS)
            nc.gpsimd.affine_select(
                out=g.a_pool, in_=g.a_pool, compare_op=ALU.is_ge, fill=0.0,
                base=0, pattern=[[-GS, 8]], channel_multiplier=1)
            nc.gpsimd.affine_select(
                out=g.a_pool, in_=g.a_pool, compare_op=ALU.is_ge, fill=0.0,
                base=GS - 1, pattern=[[GS, 8]], channel_multiplier=-1)

            g.e8 = const.tile([8, 128], BF16)
            nc.gpsimd.memset(g.e8, 1.0)
            nc.gpsimd.affine_select(
                out=g.e8, in_=g.e8, compare_op=ALU.is_ge, fill=0.0,
                base=0, pattern=[[1, 128]], channel_multiplier=-GS)
            nc.gpsimd.affine_select(
                out=g.e8, in_=g.e8, compare_op=ALU.is_ge, fill=0.0,
                base=GS - 1, pattern=[[-1, 128]], channel_multiplier=GS)

            g.ones2 = const.tile([128, 2, 16], FP8)
            nc.vector.memset(g.ones2, SCALE_V)
            g.neg2 = const.tile([128, 1], F32)
            nc.vector.memset(g.neg2, EXP_B)
            g.eps_c = const.tile([128, 1], F32)
            nc.vector.memset(g.eps_c, EPS)

            g.mq_sb = const.tile([128, 4, C], BF16)
            mqr = mq_d.ap().rearrange("(t p) d -> t p d", p=128)
            for t in range(4):
                nc.gpsimd.dma_start(g.mq_sb[:, t, :], mqr[t])
            g.wvp_sb = const.tile([128, 4, C], FP8)
            wvr = wvp_d.ap().rearrange("(t p) d -> t p d", p=128)
            for t in range(4):
                nc.gpsimd.dma_start(g.wvp_sb[:, t, :], wvr[t])

            g.gns_sb = const.tile([128, 4, 1], F32)
            nc.sync.dma_start(g.gns_sb[:, :, 0:1],
                              gns_d.ap().rearrange("(t p) -> p t", p=128))
            g.gnb_sb = const.tile([128, 4, 1], F32)
            nc.sync.dma_start(g.gnb_sb[:, :, 0:1],
                              gnb_d.ap().rearrange("(t p) -> p t", p=128))

            def bcast(src_ap):
                return bass.AP(
                    tensor=src_ap.tensor, offset=src_ap.offset,
                    ap=[[0, 128]] + [list(p) for p in src_ap.ap])

            if has_b:
                g.bres_bc = const.tile([128, 512], F32)
                nc.gpsimd.dma_start(g.bres_bc, bcast(bres_d.ap()))

            if has_u:
                uvf = const.tile([128, 4], F32)
                nc.sync.dma_start(uvf, uv_d.ap().rearrange("(t p) -> p t", p=128))
                g.uv_sb = const.tile([128, 4], BF16)
                nc.vector.tensor_copy(g.uv_sb, uvf)

            # ---- example-0 prologue -------------------------------------
            warm(40)
            xt0 = g.xt_p.tile([128, 4, 1024], BF16, tag="xt", name="xt0")
            st6_0 = g.small.tile([128, 4, 2, 6], F32, tag="st6", name="st6_0")
            for t in range(4):
                _tr_group(g, 0, xn0, xt0, st6_0, t)
            warm(8)
            pg0, vr0 = _stats_a(g, 0, st6_0)
            gab0 = _stats_rstd(g, 0, pg0, vr0)
            warm(14)
            ab0 = _stats_b(g, 0, gab0)
            warm(10)
            zt16_0 = _zt16(g, 0, xt0, ab0)
            warm(10)
            zt8_0 = _zt8_part(g, 0, zt16_0, None, (0, 1, 2, 3))
            warm(6)

            # ---- pipelined per-example emission -------------------------
            xns = {0: xn0, 1: xn1}
            state = (zt16_0, zt8_0)
            nxt = {}
            for bi in range(BPC):
                zt16, zt8 = state
                xn = xns[bi]
                if bi + 2 < BPC:
                    xns[bi + 2] = _load_x(g, bi + 2, [nc.sync])
                tr = None
                hooks = {}
                if bi + 1 < BPC:
                    xb1_t = xns[bi + 1]
                    xt1 = g.xt_p.tile([128, 4, 1024], BF16, tag="xt",
                                      name=f"xt{bi+1}")
                    st6 = g.small.tile([128, 4, 2, 6], F32, tag="st6",
                                       name=f"st6_{bi+1}")

                    def tr(m, xb1_t=xb1_t, xt1=xt1, st6=st6, b1=bi + 1):
                        _tr_group(g, b1, xb1_t, xt1, st6, m)
                gt = _g_phase(g, bi, zt16, tr)
                if bi + 1 < BPC:
                    nxt["pgvr"] = _stats_a(g, bi + 1, st6)

                    def hook2(b1=bi + 1):
                        nxt["gab"] = _stats_rstd(g, b1, *nxt["pgvr"])

                    def hook4(b1=bi + 1):
                        nxt["ab"] = _stats_b(g, b1, nxt["gab"])

                    def hook6(xt1=xt1, b1=bi + 1):
                        nxt["zt16"] = _zt16(g, b1, xt1, nxt["ab"])
                    hooks = {2: hook2, 4: hook4, 6: hook6}
                u_sb = _u_stage(g, bi, zt16) if has_u else None
                et, s_ps = _st_phase(g, bi, zt8, gt, u_sb, hooks)
                box = {}

                def after_p0(box=box, s_ps=s_ps, et=et, b=bi):
                    box["s_col"] = _denom_last(g, b, s_ps, et)
                v = _v_phase(g, bi, zt8, after_p0=after_p0)
                rc = _denom_fin(g, bi, box["s_col"])
                if bi + 1 < BPC:
                    nxt["zt8"] = _zt8_part(g, bi + 1, nxt["zt16"],
                                           None, (0, 1, 2, 3), eng="act")
                _o_phase(g, bi, xn, v, et, rc, has_b, last=(bi == BPC - 1))
                if bi + 1 < BPC:
                    state = (nxt["zt16"], nxt["zt8"])

    nc.compile()
    return nc


_NC = {}


def _get_nc(has_u, has_b):
    key = (has_u, has_b)
    if key not in _NC:
        _NC[key] = build_program(has_u, has_b)
    return _NC[key]


def kernel(x, t, gn_scale, gn_bias, w_qkv, b_qkv, w_out, b_out):
    import ml_dtypes
    x = np.ascontiguousarray(np.asarray(x, np.float32).reshape(B, N, C))
    w_qkv = np.asarray(w_qkv, np.float32)
    b_qkv = np.asarray(b_qkv, np.float32)
    w_out = np.asarray(w_out, np.float32)
    b_out = np.asarray(b_out, np.float32)
    wq, wk, wv = w_qkv[:, 0:C], w_qkv[:, C:2 * C], w_qkv[:, 2 * C:3 * C]
    bq, bv = b_qkv[0:C], b_qkv[2 * C:3 * C]

    m_qk = ((wq @ wk.T) * (ISQ * SCALE_M)).astype(ml_dtypes.bfloat16)
    w_vp = np.clip((wv @ w_out) * SCALE_V, -240.0, 240.0).astype(
        ml_dtypes.float8_e4m3)
    b_res = (bv @ w_out + b_out).astype(np.float32)
    u_vec = ((wk @ bq) * ISQ).astype(np.float32)
    has_u = bool(np.any(u_vec != 0.0))
    has_b = bool(np.any(b_res != 0.0))

    shared = {
        "m_qk": np.ascontiguousarray(m_qk),
        "w_vp": np.ascontiguousarray(w_vp),
        "gn_scale": np.ascontiguousarray(np.asarray(gn_scale, np.float32)),
        "gn_bias": np.ascontiguousarray(np.asarray(gn_bias, np.float32)),
    }
    if has_b:
        shared["b_res"] = np.ascontiguousarray(b_res)
    if has_u:
        shared["u_vec"] = np.ascontiguousarray(u_vec)
    in_maps = [
        {"x": x[c * BPC:(c + 1) * BPC], **shared} for c in range(NCORES)
    ]
    nc = _get_nc(has_u, has_b)
    res = run_bass_kernel_spmd(nc, in_maps, core_ids=list(range(NCORES)))
    out = np.concatenate([res.results[c]["out"] for c in range(NCORES)], axis=0)
    return out.reshape(B, H, W, C)
